# revision 21
# baseline (speedup 1.0000x reference)
"""DSS Linear+BN segment-reduce kernel for Trainium2, 8 NeuronCores.

Problem (N=131072, D=1024, B=2048):
    z_i = BN(x @ W_fc.T + b_fc)                      # per-element path
    x_m = segment_sum(x, seg_ids, B)                 # ragged segment sums
    x_s = BN(x_m @ W_sh.T + b_sh)                    # set path
    out = x_s[seg_ids] + z_i

Strategy (segment-aligned data parallel, one collective):
  - Host shards rows by whole segments: core c owns 256 segments chosen by
    greedy load balancing. Rows are laid out transposed (k on partitions)
    with each segment zero-padded to a multiple of 8 rows, then the core
    padded to MAX_ROWS columns. BN biases are absorbed into the BN shift.
  - Phase A: z'^T = W_fc^T-chunks .T @ x^T-chunks in bf16, bn_stats per
    tile, z'^T spilled to DRAM as bf16 (last K_CACHE blocks stay in SBUF).
    Chunk-8 partial sums of x^T are built with three bf16 "fold" adds
    (gpsimd + DVE) and PE-transposed (bf16, batched per 2 blocks) into an
    SBUF chunk-sum table s8s [chunks, 1024].
  - Middle: xm^T = s8s.T @ onehot(chunk->segment) on the PE (no indirect
    DMA), set-path matmul + bn_stats, one AllGather of packed BN sums
    [128,32], exact on-chip rank reduction, per-column scales/shifts,
    combined table C = x_s + t_fc transposed to natural layout (bf16).
  - Pass 2: out^T = z'^T * s_fc + C^T @ onehot(seg) with the one-hot built
    by is_equal on bf16 and the add fused via scalar_tensor_tensor
    (alternating DVE / gpsimd). Output written as bf16; host upcasts.
"""
import sys
import numpy as np
from contextlib import ExitStack

sys.path.insert(0, "/opt/trn_rl_repo")

import concourse.bass as bass
import concourse.bacc as bacc
import concourse.tile as tile
from concourse import mybir
from concourse.bass_utils import run_bass_kernel_spmd

F32 = mybir.dt.float32
BF16 = mybir.dt.bfloat16
I32 = mybir.dt.int32
AX = mybir.AxisListType.X
ALU = mybir.AluOpType

N, D, B, NC = 131072, 1024, 2048, 8
B_PER = B // NC            # 256 segments per core
EPS = 1e-5
CH = 8                     # segment padding / chunk size
RB = 512                   # rows per block (matmul free dim)
RB2 = 1024                 # rows per pass-2 iteration
KC = D // 128              # 8 k-chunks
DC = D // 128              # 8 d-chunks
K_CACHE = 6                # z' blocks kept in SBUF (must be even)
STT_GPSIMD = True          # alternate scalar_tensor_tensor onto gpsimd

_cache = {}


def _plan(seg_ids):
    """Host planning: per-core padded layouts + chunk->segment one-hots."""
    seg_ids = np.asarray(seg_ids)
    counts = np.bincount(seg_ids, minlength=B).astype(np.int64)
    row_start = np.zeros(B + 1, dtype=np.int64)
    np.cumsum(counts, out=row_start[1:])

    pad = ((counts + CH - 1) // CH) * CH          # padded len per segment
    # Balanced assignment: exactly B_PER segments per core (program-uniform),
    # greedily packing large segments onto the least-loaded core.
    order = np.argsort(-pad, kind="stable")
    load = np.zeros(NC, dtype=np.int64)
    nseg = np.zeros(NC, dtype=np.int64)
    assign = np.empty(B, dtype=np.int64)
    for b in order:
        cands = np.where(nseg < B_PER)[0]
        c = cands[np.argmin(load[cands])]
        assign[b] = c
        load[c] += pad[b]
        nseg[c] += 1
    max_rows = int(((load.max() + RB2 - 1) // RB2) * RB2)
    nchunk = max_rows // CH                       # multiple of 128

    plans = []
    for c in range(NC):
        segs = np.where(assign == c)[0]          # global segment ids, sorted
        cnt = counts[segs]
        pd = pad[segs]
        pstart = np.zeros(B_PER, dtype=np.int64)
        np.cumsum(pd[:-1], out=pstart[1:])
        nreal = int(cnt.sum())
        gr = np.concatenate(
            [np.arange(row_start[b], row_start[b + 1]) for b in segs]) \
            if nreal else np.empty(0, dtype=np.int64)
        local_b = np.repeat(np.arange(B_PER), cnt)
        col_ids = np.repeat(pstart, cnt) + \
            (np.arange(nreal) - np.repeat(np.cumsum(cnt) - cnt, cnt))
        nch = (pd // CH).astype(np.int64)
        used = int(nch.sum())
        chunk_seg = np.repeat(np.arange(B_PER), nch)   # [used]
        oh = np.zeros((nchunk, B_PER), dtype=np.float32)
        oh[np.arange(used), chunk_seg] = 1.0
        plans.append(dict(
            grows=gr,
            nreal=nreal,
            col_ids=col_ids,
            local_b=local_b,
            onehot=oh.reshape(nchunk // 128, 128, B_PER),
        ))
    return counts, plans, max_rows


def _build(max_rows):
    nblk = max_rows // RB
    niter = max_rows // RB2
    ngroups = max_rows // CH // 128
    kcache = min(K_CACHE, nblk)
    nunc = nblk - kcache               # uncached z' blocks (even)
    assert nunc % 2 == 0 and kcache % 2 == 0

    nc = bacc.Bacc("TRN2", target_bir_lowering=False, debug=False,
                   num_devices=NC)

    xT = nc.dram_tensor("xT", [D, max_rows], BF16, kind="ExternalInput").ap()
    sid = nc.dram_tensor("sid", [1, max_rows], BF16, kind="ExternalInput").ap()
    oh_in = nc.dram_tensor("oh", [ngroups, 128, B_PER], BF16,
                           kind="ExternalInput").ap()
    wfT = nc.dram_tensor("wfT", [D, D], BF16, kind="ExternalInput").ap()
    wsT = nc.dram_tensor("wsT", [D, D], BF16, kind="ExternalInput").ap()
    # params [128, 32]: 0:8 g_fc, 8:16 be_fc, 16:24 g_sh, 24:32 be_sh
    par = nc.dram_tensor("par", [128, 32], F32, kind="ExternalInput").ap()
    identb = nc.dram_tensor("identb", [128, 128], BF16,
                            kind="ExternalInput").ap()
    iotab = nc.dram_tensor("iotab", [128, 2], BF16, kind="ExternalInput").ap()
    outT = nc.dram_tensor("outT", [D, max_rows], BF16,
                          kind="ExternalOutput").ap()

    xT3 = xT.rearrange("(kc p) r -> p kc r", p=128)
    wfT3 = wfT.rearrange("(kc p) d -> p kc d", p=128)
    wsT3 = wsT.rearrange("(kc p) d -> p kc d", p=128)
    zT3 = lambda t: t.rearrange("(dc p) r -> p dc r", p=128)
    outT3 = outT.rearrange("(dc p) r -> p dc r", p=128)

    with tile.TileContext(nc) as tc:
        with ExitStack() as top:
            keep = top.enter_context(tc.tile_pool(name="keep", bufs=1))
            dram = top.enter_context(tc.tile_pool(name="dram", bufs=1,
                                                  space="DRAM"))

            zT = dram.tile([D, nunc * RB], BF16)
            s8d = dram.tile([ngroups, D, 128], BF16)   # transpose bounce
            d_st_in = dram.tile([128, 32], F32)
            d_st_ag = dram.tile([NC, 128, 32], F32)

            p_par = keep.tile([128, 32], F32)
            # gpsimd-triggered load: warms the gpsimd engine early so the
            # middle-phase collective trigger doesn't pay a cold start
            nc.gpsimd.dma_start(p_par[:], par[:])
            p_ident = keep.tile([128, 128], BF16)
            nc.sync.dma_start(p_ident[:], identb[:])
            p_iota = keep.tile([128, 2], BF16)
            nc.sync.dma_start(p_iota[:], iotab[:])

            bn_i = keep.tile([128, DC, nblk, 6], F32)   # element-path stats
            cn = keep.tile([128, 2, DC, 128], BF16)      # C natural chunks
            s_fc = keep.tile([128, DC], F32)             # element-path scale
            s_fcb = keep.tile([128, DC], BF16)           # bf16 copy for stt
            zcache = keep.tile([128, DC, kcache * RB], BF16)
            ei = keep.tile([128, 2, RB2], BF16)          # iota rows for is_eq

            with ExitStack() as span:
                spool = span.enter_context(tc.tile_pool(name="span", bufs=1))
                s8s = spool.tile([128, ngroups, KC, 128], BF16)
                p_oh = spool.tile([128, ngroups, B_PER], BF16)
                ws = spool.tile([128, KC, D], BF16)

                # ============ PHASE A ============
                with ExitStack() as pa:
                    wpool = pa.enter_context(tc.tile_pool(name="wf", bufs=1))
                    xpool = pa.enter_context(tc.tile_pool(name="xa", bufs=2))
                    zpool = pa.enter_context(tc.tile_pool(name="za", bufs=2))
                    fpool = pa.enter_context(tc.tile_pool(name="fa", bufs=2))
                    psA = pa.enter_context(
                        tc.tile_pool(name="psA", bufs=4, space="PSUM"))

                    wf = wpool.tile([128, KC, D], BF16)
                    nc.sync.dma_start(wf[:], wfT3)

                    s8pair = None
                    for ib in range(nblk):
                        xt = xpool.tile([128, KC, RB], BF16, tag="xt")
                        nc.sync.dma_start(
                            xt[:], xT3[:, :, ib * RB:(ib + 1) * RB])
                        if ib == 2:
                            # middle-phase inputs; loaded late so they don't
                            # delay the first x blocks on the DMA queues
                            nc.sync.dma_start(
                                p_oh[:], oh_in.rearrange("g p s -> p g s"))
                            nc.sync.dma_start(ws[:], wsT3)
                        cached = ib >= nunc
                        if cached:
                            co = (ib - nunc) * RB
                            zst = zcache[:, :, co:co + RB]
                        else:
                            zst = zpool.tile([128, DC, RB], BF16, tag="zst")
                        for dc in range(DC):
                            pz = psA.tile([128, RB], F32, tag="mm")
                            for kc in range(KC):
                                nc.tensor.matmul(
                                    pz[:], wf[:, kc, dc * 128:(dc + 1) * 128],
                                    xt[:, kc, :],
                                    start=(kc == 0), stop=(kc == KC - 1))
                            nc.vector.bn_stats(bn_i[:, dc, ib, :], pz[:])
                            nc.scalar.copy(zst[:, dc, :], pz[:])
                        if not cached:
                            nc.sync.dma_start(
                                zT3(zT)[:, :, ib * RB:(ib + 1) * RB], zst[:])

                        # chunk-8 sums via bf16 folds (gpsimd + DVE)
                        xt4 = xt[:].rearrange("p kc (c k) -> p kc c k", k=CH)
                        s4 = fpool.tile([128, KC, RB // CH, 4], BF16, tag="s4")
                        nc.vector.tensor_add(s4[:], xt4[:, :, :, 0:4],
                                             xt4[:, :, :, 4:8])
                        s2 = fpool.tile([128, KC, RB // CH, 2], BF16, tag="s2")
                        nc.vector.tensor_add(s2[:], s4[:, :, :, 0:2],
                                             s4[:, :, :, 2:4])
                        half = ib % 2
                        if half == 0:
                            s8pair = fpool.tile([128, KC, 128], BF16,
                                                tag="s8p")
                        dst = s8pair[:, :, half * 64:(half + 1) * 64] \
                            .rearrange("p kc (c o) -> p kc c o", o=1)
                        nc.vector.tensor_add(dst, s2[:, :, :, 0:1],
                                             s2[:, :, :, 1:2])
                        if half == 1:
                            # transpose chunk sums off the PE: bounce through
                            # DRAM, then xbar-transpose back into SBUF
                            g = ib // 2
                            nc.sync.dma_start(
                                s8d[g].rearrange("(kc p) c -> p kc c", p=128),
                                s8pair[:])
                            nc.sync.dma_start_transpose(
                                s8s[:, g, :, :].rearrange(
                                    "p kc f -> p (kc f)"),
                                s8d[g])

                # ============ MIDDLE ============
                with ExitStack() as pm:
                    mpool = pm.enter_context(tc.tile_pool(name="mid", bufs=1))
                    psX = pm.enter_context(
                        tc.tile_pool(name="psX", bufs=2, space="PSUM"))
                    psS = pm.enter_context(
                        tc.tile_pool(name="psS", bufs=1, space="PSUM"))
                    psT2 = pm.enter_context(
                        tc.tile_pool(name="psT2", bufs=2, space="PSUM"))

                    # xm^T directly: contract chunks against onehot
                    xmT = mpool.tile([128, KC, B_PER], BF16)
                    for kc in range(KC):
                        px = psX.tile([128, B_PER], F32, tag="xm")
                        for g in range(ngroups):
                            nc.tensor.matmul(
                                px[:], s8s[:, g, kc, :], p_oh[:, g, :],
                                start=(g == 0), stop=(g == ngroups - 1))
                        nc.vector.tensor_copy(xmT[:, kc, :], px[:])

                    # set path: zs^T [d, seg] + stats
                    zsT = mpool.tile([128, DC, B_PER], BF16)
                    bn_s = mpool.tile([128, DC, 1, 6], F32)
                    for dc in range(DC):
                        pzs = psS.tile([128, B_PER], F32, tag="set", bufs=2)
                        for kc in range(KC):
                            nc.tensor.matmul(
                                pzs[:], ws[:, kc, dc * 128:(dc + 1) * 128],
                                xmT[:, kc, :],
                                start=(kc == 0), stop=(kc == KC - 1))
                        nc.vector.bn_stats(bn_s[:, dc, 0, :], pzs[:])
                        nc.vector.tensor_copy(zsT[:, dc, :], pzs[:])

                    # pack local sums: [0:8] sum_i, [8:16] sumsq_i,
                    # [16:24] sum_s, [24:32] sumsq_s
                    loc = mpool.tile([128, 32], F32)
                    mv_i = mpool.tile([128, DC, 2], F32)
                    mv_s = mpool.tile([128, DC, 2], F32)
                    for dc in range(DC):
                        nc.vector.bn_aggr(mv_i[:, dc, :], bn_i[:, dc, :, :])
                        nc.vector.bn_aggr(mv_s[:, dc, :], bn_s[:, dc, :, :])
                    tmp = mpool.tile([128, DC], F32)
                    for (mv, cnt_, o_s, o_q) in (
                            (mv_i, float(max_rows), 0, 8),
                            (mv_s, float(B_PER), 16, 24)):
                        nc.vector.tensor_scalar_mul(
                            loc[:, o_s:o_s + 8], mv[:, :, 0], cnt_)
                        nc.vector.tensor_mul(tmp[:], mv[:, :, 0], mv[:, :, 0])
                        nc.vector.tensor_add(tmp[:], tmp[:], mv[:, :, 1])
                        nc.vector.tensor_scalar_mul(
                            loc[:, o_q:o_q + 8], tmp[:], cnt_)

                    # stats-independent prep, before the collective blocks
                    # the queues: iota rows for pass-2 is_eq
                    for h in range(2):
                        nc.vector.tensor_copy(
                            ei[:, h, :],
                            p_iota[:, h:h + 1].to_broadcast([128, RB2]))

                    nc.sync.dma_start(d_st_in[:], loc[:])
                    nc.gpsimd.collective_compute(
                        "AllGather", ALU.bypass,
                        replica_groups=[list(range(NC))],
                        ins=[d_st_in[:].opt()], outs=[d_st_ag[:].opt()])
                    rk = mpool.tile([128, NC, 32], F32)
                    nc.sync.dma_start(rk[:], d_st_ag.rearrange("r p j -> p r j"))
                    g32 = mpool.tile([128, 32], F32)
                    nc.vector.reduce_sum(
                        out=g32[:], in_=rk[:].rearrange("p r j -> p j r"),
                        axis=AX)

                    # scales/shifts per d-column ([128, 8] transposed layout)
                    def bn_affine(sum_sl, sq_sl, inv_n, g_sl, be_sl, s_out,
                                  sfx):
                        m = mpool.tile([128, DC], F32, tag=f"m{sfx}")
                        nc.vector.tensor_scalar_mul(m[:], g32[:, sum_sl],
                                                    inv_n)
                        v = mpool.tile([128, DC], F32, tag=f"v{sfx}")
                        nc.vector.tensor_scalar_mul(v[:], g32[:, sq_sl],
                                                    inv_n)
                        t2 = mpool.tile([128, DC], F32, tag=f"t2{sfx}")
                        nc.vector.tensor_mul(t2[:], m[:], m[:])
                        nc.vector.tensor_sub(v[:], v[:], t2[:])
                        nc.vector.tensor_scalar_add(v[:], v[:], EPS)
                        nc.scalar.sqrt(v[:], v[:])
                        nc.vector.reciprocal(v[:], v[:])
                        nc.vector.tensor_mul(s_out[:], v[:], p_par[:, g_sl])
                        t_out = mpool.tile([128, DC], F32, tag=f"t{sfx}")
                        nc.vector.tensor_mul(t_out[:], m[:], s_out[:])
                        nc.vector.tensor_sub(t_out[:], p_par[:, be_sl],
                                             t_out[:])
                        return t_out

                    t_fc = bn_affine(slice(0, 8), slice(8, 16), 1.0 / N,
                                     slice(0, 8), slice(8, 16), s_fc, "i")
                    s_sh = mpool.tile([128, DC], F32)
                    t_sh = bn_affine(slice(16, 24), slice(24, 32), 1.0 / B,
                                     slice(16, 24), slice(24, 32), s_sh, "s")
                    tb = mpool.tile([128, DC], F32)
                    nc.vector.tensor_add(tb[:], t_sh[:], t_fc[:])
                    nc.vector.tensor_copy(s_fcb[:], s_fc[:])

                    # C^T = zs^T * s_sh + (t_sh + t_fc); transpose to natural
                    ct = mpool.tile([128, DC, B_PER], BF16)
                    for dc in range(DC):
                        nc.vector.tensor_scalar(
                            out=ct[:, dc, :], in0=zsT[:, dc, :],
                            scalar1=s_sh[:, dc:dc + 1],
                            scalar2=tb[:, dc:dc + 1],
                            op0=ALU.mult, op1=ALU.add)
                    for h in range(2):
                        for dc in range(DC):
                            pt = psT2.tile([128, 128], BF16, tag="tr2")
                            nc.tensor.transpose(
                                pt[:], ct[:, dc, h * 128:(h + 1) * 128],
                                p_ident[:])
                            nc.vector.tensor_copy(cn[:, h, dc, :], pt[:])



            # ============ PASS 2 ============
            with ExitStack() as p2:
                z2pool = p2.enter_context(tc.tile_pool(name="z2", bufs=3))
                opool = p2.enter_context(tc.tile_pool(name="o2", bufs=3))
                epool = p2.enter_context(tc.tile_pool(name="e2", bufs=3))
                cpool = p2.enter_context(tc.tile_pool(name="c2", bufs=4))
                ps2 = p2.enter_context(
                    tc.tile_pool(name="ps2", bufs=3, space="PSUM"))

                for it in range(niter):
                    cached = it >= nunc // 2
                    if cached:
                        co = it * RB2 - nunc * RB
                        zt = zcache[:, :, co:co + RB2]
                    else:
                        zt = z2pool.tile([128, DC, RB2], BF16, tag="zt")
                        nc.sync.dma_start(
                            zt[:], zT3(zT)[:, :, it * RB2:(it + 1) * RB2])
                    sidb = epool.tile([128, RB2], BF16, tag="sid")
                    nc.sync.dma_start(
                        sidb[:],
                        sid[:1, it * RB2:(it + 1) * RB2]
                        .to_broadcast([128, RB2]))
                    e0 = epool.tile([128, RB2], BF16, tag="e0")
                    e1 = epool.tile([128, RB2], BF16, tag="e1")
                    nc.vector.tensor_tensor(
                        out=e0[:], in0=ei[:, 0, :], in1=sidb[:],
                        op=ALU.is_equal)
                    nc.vector.tensor_tensor(
                        out=e1[:], in0=ei[:, 1, :], in1=sidb[:],
                        op=ALU.is_equal)
                    ob = opool.tile([128, DC, RB2], BF16, tag="ob")
                    for dc in range(DC):
                        px = ps2.tile([128, RB2], F32, tag="ex")
                        for q in range(2):
                            nc.tensor.matmul(
                                px[:, q * RB:(q + 1) * RB], cn[:, 0, dc, :],
                                e0[:, q * RB:(q + 1) * RB],
                                start=True, stop=False)
                            nc.tensor.matmul(
                                px[:, q * RB:(q + 1) * RB], cn[:, 1, dc, :],
                                e1[:, q * RB:(q + 1) * RB],
                                start=False, stop=True)
                        pc = cpool.tile([128, RB2], BF16, tag="pc")
                        nc.scalar.copy(pc[:], px[:])
                        nc.vector.scalar_tensor_tensor(
                            out=ob[:, dc, :],
                            in0=zt[:, dc, :],
                            scalar=s_fcb[:, dc:dc + 1],
                            in1=pc[:],
                            op0=ALU.mult, op1=ALU.add)
                    nc.sync.dma_start(
                        outT3[:, :, it * RB2:(it + 1) * RB2], ob[:])

    nc.compile()
    return nc


def kernel(x, W_fc, b_fc, g_fc, be_fc, W_sh, b_sh, g_sh, be_sh, seg_ids,
           _want_trace=False):
    x = np.ascontiguousarray(np.asarray(x, dtype=np.float32))
    seg_ids = np.asarray(seg_ids, dtype=np.int32)
    counts, plans, max_rows = _plan(seg_ids)

    key = (max_rows,)
    if key not in _cache:
        _cache[key] = _build(max_rows)
    nc = _cache[key]

    import ml_dtypes
    bf = ml_dtypes.bfloat16
    wfT = np.ascontiguousarray(np.asarray(W_fc, np.float32).T).astype(bf)
    wsT = np.ascontiguousarray(np.asarray(W_sh, np.float32).T).astype(bf)
    par = np.zeros((128, 32), dtype=np.float32)
    par[:, 0:8] = np.asarray(g_fc, np.float32).reshape(8, 128).T
    par[:, 8:16] = np.asarray(be_fc, np.float32).reshape(8, 128).T
    par[:, 16:24] = np.asarray(g_sh, np.float32).reshape(8, 128).T
    par[:, 24:32] = np.asarray(be_sh, np.float32).reshape(8, 128).T
    identb = np.eye(128, dtype=np.float32).astype(bf)
    iotab = np.stack([np.arange(128, dtype=np.float32),
                      np.arange(128, 256, dtype=np.float32)],
                     axis=1).astype(bf)

    in_maps = []
    for c in range(NC):
        p = plans[c]
        xp = np.zeros((max_rows, D), dtype=bf)
        xp[p["col_ids"]] = x[p["grows"]].astype(bf)
        sid_row = np.full((1, max_rows), 999.0, dtype=bf)
        sid_row[0, p["col_ids"]] = p["local_b"].astype(bf)
        in_maps.append(dict(
            xT=np.ascontiguousarray(xp.T), sid=sid_row,
            oh=p["onehot"].astype(bf),
            wfT=wfT, wsT=wsT, par=par, identb=identb, iotab=iotab))

    kw = {}
    if _want_trace:
        kw = dict(trace=True)
    res = run_bass_kernel_spmd(nc, in_maps, core_ids=list(range(NC)), **kw)

    out = np.empty((N, D), dtype=np.float32)
    for c in range(NC):
        p = plans[c]
        oT = res.results[c]["outT"]          # [D, max_rows] bf16
        out[p["grows"]] = oT.T[p["col_ids"]].astype(np.float32)
    if _want_trace:
        return out, res
    return out


# revision 33
# speedup vs baseline: 1.0120x; 1.0120x over previous
"""DSS Linear+BN segment-reduce kernel for Trainium2, 8 NeuronCores.

Problem (N=131072, D=1024, B=2048):
    z_i = BN(x @ W_fc.T + b_fc)                      # per-element path
    x_m = segment_sum(x, seg_ids, B)                 # ragged segment sums
    x_s = BN(x_m @ W_sh.T + b_sh)                    # set path
    out = x_s[seg_ids] + z_i

Strategy (segment-aligned data parallel, one collective):
  - Host shards rows by whole segments: core c owns 256 segments chosen by
    greedy load balancing. Rows are laid out transposed (k on partitions)
    with each segment zero-padded to a multiple of 8 rows, then the core
    padded to MAX_ROWS columns. BN biases are absorbed into the BN shift.
  - Phase A: z'^T = W_fc^T-chunks .T @ x^T-chunks in bf16, bn_stats per
    tile, z'^T spilled to DRAM as bf16 (last K_CACHE blocks stay in SBUF).
    Chunk-8 partial sums of x^T are built with three bf16 "fold" adds
    (gpsimd + DVE) and PE-transposed (bf16, batched per 2 blocks) into an
    SBUF chunk-sum table s8s [chunks, 1024].
  - Middle: xm^T = s8s.T @ onehot(chunk->segment) on the PE (no indirect
    DMA), set-path matmul + bn_stats, one AllGather of packed BN sums
    [128,32], exact on-chip rank reduction, per-column scales/shifts,
    combined table C = x_s + t_fc transposed to natural layout (bf16).
  - Pass 2: out^T = z'^T * s_fc + C^T @ onehot(seg) with the one-hot built
    by is_equal on bf16 and the add fused via scalar_tensor_tensor
    (alternating DVE / gpsimd). Output written as bf16; host upcasts.
"""
import sys
import numpy as np
from contextlib import ExitStack

sys.path.insert(0, "/opt/trn_rl_repo")

import concourse.bass as bass
import concourse.bacc as bacc
import concourse.tile as tile
from concourse import mybir
from concourse.bass_utils import run_bass_kernel_spmd

F32 = mybir.dt.float32
BF16 = mybir.dt.bfloat16
I32 = mybir.dt.int32
AX = mybir.AxisListType.X
ALU = mybir.AluOpType

N, D, B, NC = 131072, 1024, 2048, 8
B_PER = B // NC            # 256 segments per core
EPS = 1e-5
CH = 8                     # segment padding / chunk size
RB = 512                   # rows per block (matmul free dim)
RB2 = 1024                 # rows per pass-2 iteration
KC = D // 128              # 8 k-chunks
DC = D // 128              # 8 d-chunks
K_CACHE = 4                # z' blocks kept in SBUF (must be even)

_cache = {}


def _plan(seg_ids):
    """Host planning: per-core padded layouts + chunk->segment one-hots."""
    seg_ids = np.asarray(seg_ids)
    counts = np.bincount(seg_ids, minlength=B).astype(np.int64)
    row_start = np.zeros(B + 1, dtype=np.int64)
    np.cumsum(counts, out=row_start[1:])

    pad = ((counts + CH - 1) // CH) * CH          # padded len per segment
    # Balanced assignment: exactly B_PER segments per core (program-uniform),
    # greedily packing large segments onto the least-loaded core.
    order = np.argsort(-pad, kind="stable")
    load = np.zeros(NC, dtype=np.int64)
    nseg = np.zeros(NC, dtype=np.int64)
    assign = np.empty(B, dtype=np.int64)
    for b in order:
        cands = np.where(nseg < B_PER)[0]
        c = cands[np.argmin(load[cands])]
        assign[b] = c
        load[c] += pad[b]
        nseg[c] += 1
    max_rows = int(((load.max() + RB2 - 1) // RB2) * RB2)
    nchunk = max_rows // CH                       # multiple of 128

    plans = []
    for c in range(NC):
        segs = np.where(assign == c)[0]          # global segment ids, sorted
        cnt = counts[segs]
        pd = pad[segs]
        pstart = np.zeros(B_PER, dtype=np.int64)
        np.cumsum(pd[:-1], out=pstart[1:])
        nreal = int(cnt.sum())
        gr = np.concatenate(
            [np.arange(row_start[b], row_start[b + 1]) for b in segs]) \
            if nreal else np.empty(0, dtype=np.int64)
        local_b = np.repeat(np.arange(B_PER), cnt)
        col_ids = np.repeat(pstart, cnt) + \
            (np.arange(nreal) - np.repeat(np.cumsum(cnt) - cnt, cnt))
        nch = (pd // CH).astype(np.int64)
        used = int(nch.sum())
        chunk_seg = np.repeat(np.arange(B_PER), nch)   # [used]
        oh = np.zeros((nchunk, B_PER), dtype=np.float32)
        oh[np.arange(used), chunk_seg] = 1.0
        plans.append(dict(
            grows=gr,
            nreal=nreal,
            col_ids=col_ids,
            local_b=local_b,
            onehot=oh.reshape(nchunk // 128, 128, B_PER),
        ))
    return counts, plans, max_rows


def _build(max_rows):
    nblk = max_rows // RB
    niter = max_rows // RB2
    ngroups = max_rows // CH // 128
    kcache = min(K_CACHE, nblk)
    nunc = nblk - kcache               # uncached z' blocks (even)
    assert nunc % 2 == 0 and kcache % 2 == 0

    nc = bacc.Bacc("TRN2", target_bir_lowering=False, debug=False,
                   num_devices=NC)

    # x blocks in [block, partition, kc*r] layout: every bulk DMA is one
    # contiguous run per partition (descriptor generation is the sync-queue
    # bottleneck otherwise)
    xT = nc.dram_tensor("xT", [nblk, 128, KC * RB], BF16,
                        kind="ExternalInput").ap()
    sid = nc.dram_tensor("sid", [128, max_rows], BF16,
                         kind="ExternalInput").ap()
    oh_in = nc.dram_tensor("oh", [ngroups, 128, B_PER], BF16,
                           kind="ExternalInput").ap()
    wfT = nc.dram_tensor("wfT", [D, D], BF16, kind="ExternalInput").ap()
    wsT = nc.dram_tensor("wsT", [D, D], BF16, kind="ExternalInput").ap()
    # params [128, 32]: 0:8 g_fc, 8:16 be_fc, 16:24 g_sh, 24:32 be_sh
    par = nc.dram_tensor("par", [128, 32], F32, kind="ExternalInput").ap()
    identb = nc.dram_tensor("identb", [128, 128], BF16,
                            kind="ExternalInput").ap()
    iotab = nc.dram_tensor("iotab", [128, 2], BF16, kind="ExternalInput").ap()
    outT = nc.dram_tensor("outT", [niter, 128, DC * RB2], BF16,
                          kind="ExternalOutput").ap()

    wfT3 = wfT.rearrange("(kc p) d -> p kc d", p=128)
    wsT3 = wsT.rearrange("(kc p) d -> p kc d", p=128)

    with tile.TileContext(nc) as tc:
        with ExitStack() as top:
            keep = top.enter_context(tc.tile_pool(name="keep", bufs=1))
            dram = top.enter_context(tc.tile_pool(name="dram", bufs=1,
                                                  space="DRAM"))

            zT = dram.tile([nunc // 2, 128, DC * RB2], BF16)
            s8d = dram.tile([ngroups, D, 128], BF16)   # transpose bounce
            d_st_in = dram.tile([128, 32], F32)
            d_st_ag = dram.tile([NC, 128, 32], F32)

            p_par = keep.tile([128, 32], F32)
            # gpsimd-triggered load: warms the gpsimd engine early so the
            # middle-phase collective trigger doesn't pay a cold start
            nc.gpsimd.dma_start(p_par[:], par[:])
            p_ident = keep.tile([128, 128], BF16)
            nc.sync.dma_start(p_ident[:], identb[:])
            p_iota = keep.tile([128, 2], BF16)
            nc.sync.dma_start(p_iota[:], iotab[:])

            bn_i = keep.tile([128, DC, nblk, 6], F32)   # element-path stats
            cn = keep.tile([128, 2, DC, 128], BF16)      # C natural chunks
            s_fc = keep.tile([128, DC], F32)             # element-path scale
            s_fcb = keep.tile([128, DC], BF16)           # bf16 copy for stt
            zcache = keep.tile([128, DC, kcache * RB], BF16)
            ei = keep.tile([128, 2, RB2], BF16)          # iota rows for is_eq

            with ExitStack() as span:
                spool = span.enter_context(tc.tile_pool(name="span", bufs=1))
                s8s = [spool.tile([128, KC, 128], BF16, name=f"s8s{g}")
                       for g in range(ngroups)]
                p_oh = spool.tile([128, ngroups, B_PER], BF16)
                ws = spool.tile([128, KC, D], BF16)

                # ============ PHASE A ============
                with ExitStack() as pa:
                    wpool = pa.enter_context(tc.tile_pool(name="wf", bufs=1))
                    xpool = pa.enter_context(tc.tile_pool(name="xa", bufs=2))
                    zpool = pa.enter_context(tc.tile_pool(name="za", bufs=2))
                    fpool = pa.enter_context(tc.tile_pool(name="fa", bufs=2))
                    psA = pa.enter_context(
                        tc.tile_pool(name="psA", bufs=4, space="PSUM"))

                    wf = wpool.tile([128, KC, D], BF16)
                    nc.sync.dma_start(wf[:], wfT3)

                    s8pair = None
                    zpair = None
                    for ib in range(nblk):
                        xt = xpool.tile([128, KC, RB], BF16, tag="xt")
                        nc.sync.dma_start(
                            xt[:],
                            xT[ib].rearrange("p (kc r) -> p kc r", r=RB))
                        if ib == 2:
                            # middle-phase inputs; loaded late so they don't
                            # delay the first x blocks on the DMA queues
                            nc.sync.dma_start(
                                p_oh[:], oh_in.rearrange("g p s -> p g s"))
                            nc.sync.dma_start(ws[:], wsT3)
                        half = ib % 2
                        cached = ib >= nunc
                        if cached:
                            co = (ib - nunc) * RB
                            zst = zcache[:, :, co:co + RB]
                        else:
                            if half == 0:
                                zpair = zpool.tile([128, DC, RB2], BF16,
                                                   tag="zp")
                            zst = zpair[:, :, half * RB:(half + 1) * RB]
                        for dc in range(DC):
                            pz = psA.tile([128, RB], F32, tag="mm")
                            for kc in range(KC):
                                nc.tensor.matmul(
                                    pz[:], wf[:, kc, dc * 128:(dc + 1) * 128],
                                    xt[:, kc, :],
                                    start=(kc == 0), stop=(kc == KC - 1))
                            nc.vector.bn_stats(bn_i[:, dc, ib, :], pz[:])
                            nc.scalar.copy(zst[:, dc, :], pz[:])
                        if not cached and half == 1:
                            nc.sync.dma_start(
                                zT[ib // 2].rearrange(
                                    "p (dc r) -> p dc r", r=RB2),
                                zpair[:])

                        # chunk-8 sums via bf16 folds (gpsimd + DVE)
                        xt4 = xt[:].rearrange("p kc (c k) -> p kc c k", k=CH)
                        s4 = fpool.tile([128, KC, RB // CH, 4], BF16, tag="s4")
                        nc.vector.tensor_add(s4[:], xt4[:, :, :, 0:4],
                                             xt4[:, :, :, 4:8])
                        s2 = fpool.tile([128, KC, RB // CH, 2], BF16, tag="s2")
                        nc.vector.tensor_add(s2[:], s4[:, :, :, 0:2],
                                             s4[:, :, :, 2:4])
                        if half == 0:
                            s8pair = fpool.tile([128, KC, 128], BF16,
                                                tag="s8p")
                        dst = s8pair[:, :, half * 64:(half + 1) * 64] \
                            .rearrange("p kc (c o) -> p kc c o", o=1)
                        nc.vector.tensor_add(dst, s2[:, :, :, 0:1],
                                             s2[:, :, :, 1:2])
                        if half == 1:
                            # transpose chunk sums off the PE: bounce through
                            # DRAM, then xbar-transpose back into SBUF
                            g = ib // 2
                            nc.sync.dma_start(
                                s8d[g].rearrange("(kc p) c -> p kc c", p=128),
                                s8pair[:])
                            nc.sync.dma_start_transpose(
                                s8s[g][:].rearrange("p kc f -> p (kc f)"),
                                s8d[g])

                # ============ MIDDLE ============
                with ExitStack() as pm:
                    mpool = pm.enter_context(tc.tile_pool(name="mid", bufs=1))
                    psX = pm.enter_context(
                        tc.tile_pool(name="psX", bufs=2, space="PSUM"))
                    psS = pm.enter_context(
                        tc.tile_pool(name="psS", bufs=1, space="PSUM"))
                    psT2 = pm.enter_context(
                        tc.tile_pool(name="psT2", bufs=2, space="PSUM"))

                    # xm^T directly: contract chunks against onehot
                    xmT = mpool.tile([128, KC, B_PER], BF16)
                    for kc in range(KC):
                        px = psX.tile([128, B_PER], F32, tag="xm")
                        for g in range(ngroups):
                            nc.tensor.matmul(
                                px[:], s8s[g][:, kc, :], p_oh[:, g, :],
                                start=(g == 0), stop=(g == ngroups - 1))
                        nc.vector.tensor_copy(xmT[:, kc, :], px[:])

                    # set path: zs^T [d, seg] + stats
                    zsT = mpool.tile([128, DC, B_PER], BF16)
                    bn_s = mpool.tile([128, DC, 1, 6], F32)
                    for dc in range(DC):
                        pzs = psS.tile([128, B_PER], F32, tag="set", bufs=2)
                        for kc in range(KC):
                            nc.tensor.matmul(
                                pzs[:], ws[:, kc, dc * 128:(dc + 1) * 128],
                                xmT[:, kc, :],
                                start=(kc == 0), stop=(kc == KC - 1))
                        nc.vector.bn_stats(bn_s[:, dc, 0, :], pzs[:])
                        nc.vector.tensor_copy(zsT[:, dc, :], pzs[:])

                    # pack local sums: [0:8] sum_i, [8:16] sumsq_i,
                    # [16:24] sum_s, [24:32] sumsq_s
                    loc = mpool.tile([128, 32], F32)
                    mv_i = mpool.tile([128, DC, 2], F32)
                    mv_s = mpool.tile([128, DC, 2], F32)
                    for dc in range(DC):
                        nc.vector.bn_aggr(mv_i[:, dc, :], bn_i[:, dc, :, :])
                        nc.vector.bn_aggr(mv_s[:, dc, :], bn_s[:, dc, :, :])
                    tmp = mpool.tile([128, DC], F32)
                    for (mv, cnt_, o_s, o_q) in (
                            (mv_i, float(max_rows), 0, 8),
                            (mv_s, float(B_PER), 16, 24)):
                        nc.vector.tensor_scalar_mul(
                            loc[:, o_s:o_s + 8], mv[:, :, 0], cnt_)
                        nc.vector.tensor_mul(tmp[:], mv[:, :, 0], mv[:, :, 0])
                        nc.vector.tensor_add(tmp[:], tmp[:], mv[:, :, 1])
                        nc.vector.tensor_scalar_mul(
                            loc[:, o_q:o_q + 8], tmp[:], cnt_)

                    # stats-independent prep, before the collective blocks
                    # the queues: iota rows for pass-2 is_eq
                    for h in range(2):
                        nc.vector.tensor_copy(
                            ei[:, h, :],
                            p_iota[:, h:h + 1].to_broadcast([128, RB2]))

                    nc.sync.dma_start(d_st_in[:], loc[:])
                    nc.gpsimd.collective_compute(
                        "AllGather", ALU.bypass,
                        replica_groups=[list(range(NC))],
                        ins=[d_st_in[:].opt()], outs=[d_st_ag[:].opt()])
                    rk = mpool.tile([128, NC, 32], F32)
                    nc.sync.dma_start(rk[:], d_st_ag.rearrange("r p j -> p r j"))
                    g32 = mpool.tile([128, 32], F32)
                    nc.vector.reduce_sum(
                        out=g32[:], in_=rk[:].rearrange("p r j -> p j r"),
                        axis=AX)

                    # scales/shifts per d-column ([128, 8] transposed layout)
                    def bn_affine(sum_sl, sq_sl, inv_n, g_sl, be_sl, s_out,
                                  sfx):
                        m = mpool.tile([128, DC], F32, tag=f"m{sfx}")
                        nc.vector.tensor_scalar_mul(m[:], g32[:, sum_sl],
                                                    inv_n)
                        v = mpool.tile([128, DC], F32, tag=f"v{sfx}")
                        nc.vector.tensor_scalar_mul(v[:], g32[:, sq_sl],
                                                    inv_n)
                        t2 = mpool.tile([128, DC], F32, tag=f"t2{sfx}")
                        nc.vector.tensor_mul(t2[:], m[:], m[:])
                        nc.vector.tensor_sub(v[:], v[:], t2[:])
                        nc.vector.tensor_scalar_add(v[:], v[:], EPS)
                        nc.scalar.sqrt(v[:], v[:])
                        nc.vector.reciprocal(v[:], v[:])
                        nc.vector.tensor_mul(s_out[:], v[:], p_par[:, g_sl])
                        t_out = mpool.tile([128, DC], F32, tag=f"t{sfx}")
                        nc.vector.tensor_mul(t_out[:], m[:], s_out[:])
                        nc.vector.tensor_sub(t_out[:], p_par[:, be_sl],
                                             t_out[:])
                        return t_out

                    t_fc = bn_affine(slice(0, 8), slice(8, 16), 1.0 / N,
                                     slice(0, 8), slice(8, 16), s_fc, "i")
                    s_sh = mpool.tile([128, DC], F32)
                    t_sh = bn_affine(slice(16, 24), slice(24, 32), 1.0 / B,
                                     slice(16, 24), slice(24, 32), s_sh, "s")
                    tb = mpool.tile([128, DC], F32)
                    nc.vector.tensor_add(tb[:], t_sh[:], t_fc[:])
                    nc.vector.tensor_copy(s_fcb[:], s_fc[:])

                    # C^T = zs^T * s_sh + (t_sh + t_fc); transpose to natural
                    ct = mpool.tile([128, DC, B_PER], BF16)
                    for dc in range(DC):
                        nc.vector.tensor_scalar(
                            out=ct[:, dc, :], in0=zsT[:, dc, :],
                            scalar1=s_sh[:, dc:dc + 1],
                            scalar2=tb[:, dc:dc + 1],
                            op0=ALU.mult, op1=ALU.add)
                    for h in range(2):
                        for dc in range(DC):
                            pt = psT2.tile([128, 128], BF16, tag="tr2")
                            nc.tensor.transpose(
                                pt[:], ct[:, dc, h * 128:(h + 1) * 128],
                                p_ident[:])
                            nc.vector.tensor_copy(cn[:, h, dc, :], pt[:])



            # ============ PASS 2 ============
            with ExitStack() as p2:
                z2pool = p2.enter_context(tc.tile_pool(name="z2", bufs=3))
                opool = p2.enter_context(tc.tile_pool(name="o2", bufs=3))
                epool = p2.enter_context(tc.tile_pool(name="e2", bufs=3))
                cpool = p2.enter_context(tc.tile_pool(name="c2", bufs=4))
                ps2 = p2.enter_context(
                    tc.tile_pool(name="ps2", bufs=3, space="PSUM"))

                for it in range(niter):
                    cached = it >= nunc // 2
                    if cached:
                        co = it * RB2 - nunc * RB
                        zt = zcache[:, :, co:co + RB2]
                    else:
                        zt = z2pool.tile([128, DC, RB2], BF16, tag="zt")
                        nc.sync.dma_start(
                            zt[:],
                            zT[it].rearrange("p (dc r) -> p dc r", r=RB2))
                    sidb = epool.tile([128, RB2], BF16, tag="sid")
                    nc.sync.dma_start(
                        sidb[:], sid[:, it * RB2:(it + 1) * RB2])
                    e0 = epool.tile([128, RB2], BF16, tag="e0")
                    e1 = epool.tile([128, RB2], BF16, tag="e1")
                    nc.vector.tensor_tensor(
                        out=e0[:], in0=ei[:, 0, :], in1=sidb[:],
                        op=ALU.is_equal)
                    nc.vector.tensor_tensor(
                        out=e1[:], in0=ei[:, 1, :], in1=sidb[:],
                        op=ALU.is_equal)
                    ob = opool.tile([128, DC, RB2], BF16, tag="ob")
                    for dc in range(DC):
                        px = ps2.tile([128, RB2], F32, tag="ex")
                        for q in range(2):
                            nc.tensor.matmul(
                                px[:, q * RB:(q + 1) * RB], cn[:, 0, dc, :],
                                e0[:, q * RB:(q + 1) * RB],
                                start=True, stop=False)
                            nc.tensor.matmul(
                                px[:, q * RB:(q + 1) * RB], cn[:, 1, dc, :],
                                e1[:, q * RB:(q + 1) * RB],
                                start=False, stop=True)
                        pc = cpool.tile([128, RB2], BF16, tag="pc")
                        nc.scalar.copy(pc[:], px[:])
                        nc.vector.scalar_tensor_tensor(
                            out=ob[:, dc, :],
                            in0=zt[:, dc, :],
                            scalar=s_fcb[:, dc:dc + 1],
                            in1=pc[:],
                            op0=ALU.mult, op1=ALU.add)
                    nc.sync.dma_start(
                        outT[it].rearrange("p (dc r) -> p dc r", r=RB2),
                        ob[:])

    nc.compile()
    return nc


def kernel(x, W_fc, b_fc, g_fc, be_fc, W_sh, b_sh, g_sh, be_sh, seg_ids,
           _want_trace=False):
    x = np.ascontiguousarray(np.asarray(x, dtype=np.float32))
    seg_ids = np.asarray(seg_ids, dtype=np.int32)
    counts, plans, max_rows = _plan(seg_ids)

    key = (max_rows,)
    if key not in _cache:
        _cache[key] = _build(max_rows)
    nc = _cache[key]

    import ml_dtypes
    bf = ml_dtypes.bfloat16
    wfT = np.ascontiguousarray(np.asarray(W_fc, np.float32).T).astype(bf)
    wsT = np.ascontiguousarray(np.asarray(W_sh, np.float32).T).astype(bf)
    par = np.zeros((128, 32), dtype=np.float32)
    par[:, 0:8] = np.asarray(g_fc, np.float32).reshape(8, 128).T
    par[:, 8:16] = np.asarray(be_fc, np.float32).reshape(8, 128).T
    par[:, 16:24] = np.asarray(g_sh, np.float32).reshape(8, 128).T
    par[:, 24:32] = np.asarray(be_sh, np.float32).reshape(8, 128).T
    identb = np.eye(128, dtype=np.float32).astype(bf)
    iotab = np.stack([np.arange(128, dtype=np.float32),
                      np.arange(128, 256, dtype=np.float32)],
                     axis=1).astype(bf)

    nblk = max_rows // RB
    niter = max_rows // RB2
    in_maps = []
    for c in range(NC):
        p = plans[c]
        xp = np.zeros((max_rows, D), dtype=bf)
        xp[p["col_ids"]] = x[p["grows"]].astype(bf)
        # [block, partition(k%128), kc*r]: one contiguous run per partition
        xTb = np.ascontiguousarray(
            xp.reshape(nblk, RB, KC, 128).transpose(0, 3, 2, 1)
            .reshape(nblk, 128, KC * RB))
        sid_row = np.full((max_rows,), 999.0, dtype=bf)
        sid_row[p["col_ids"]] = p["local_b"].astype(bf)
        sid_rep = np.ascontiguousarray(
            np.broadcast_to(sid_row[None, :], (128, max_rows)))
        in_maps.append(dict(
            xT=xTb, sid=sid_rep,
            oh=p["onehot"].astype(bf),
            wfT=wfT, wsT=wsT, par=par, identb=identb, iotab=iotab))

    kw = {}
    if _want_trace:
        kw = dict(trace=True)
    res = run_bass_kernel_spmd(nc, in_maps, core_ids=list(range(NC)), **kw)

    out = np.empty((N, D), dtype=np.float32)
    for c in range(NC):
        p = plans[c]
        ob = np.asarray(res.results[c]["outT"])   # [niter, 128, DC*RB2] bf16
        # element [it, p, dc, r] = out_T[dc*128+p, it*RB2+r]
        rows = ob.reshape(niter, 128, DC, RB2).transpose(0, 3, 2, 1) \
            .reshape(max_rows, D)                 # [col, d]
        out[p["grows"]] = rows[p["col_ids"]].astype(np.float32)
    if _want_trace:
        return out, res
    return out
